# revision 17
# baseline (speedup 1.0000x reference)
"""3-layer GCN (DiffPool-style conv stack) on Trainium2, 8 NeuronCores.

Strategy (graph/data parallel, per sharding hint):
  - Nodes are permuted by degree and dealt round-robin to 8 cores
    (12544 local nodes each incl. dummy padding; 98 blocks of 128).
  - Edges partitioned by destination owner; per core the edge stream is
    grouped by (src quadrant, dst block) so dma_gather indices fit int16
    and each 128-edge tile maps to a single static PSUM block.
  - Per layer: each core computes the table rows for its own nodes
    T = dinv * (H @ W) (node-major), AllGather assembles the full table,
    then per-edge rows are fetched with gpsimd dma_gather (<=1024
    idxs/call, 4 SWDGE queues) and aggregated with one-hot selection
    matmuls into per-block PSUM, accumulated across quadrants in an SBUF
    slab, then scaled/biased/relu'd into the next layer's input.
"""

import sys
import types

sys.path.insert(0, "/opt/trn_rl_repo")

import numpy as np

N = 100000
C = 128
NC = 8
L = 12544           # local nodes per core (98 blocks of 128)
B = L // 128        # 98
NPAD = NC * L       # 100352
QUADS = 4
QROWS = NPAD // QUADS   # 25088 (< 32767, fits int16 gather index)
# dma_gather ucode limit: 1024 indices per call (8 tiles of 128).
CALL_MAX_TILES = 8
N_QUEUES = 4
USE_SHARED_TABLES = True

import ml_dtypes

TBL_NP = ml_dtypes.bfloat16  # table dtype; np.float32 or ml_dtypes.bfloat16


def _install_axon_profile_hook():
    """run_bass_kernel_spmd(trace=True) needs antenv.axon_hooks, absent in
    this image; register the equivalent ctypes hook."""
    try:
        import antenv
        if getattr(antenv, "axon_hooks", None) is not None:
            return
        from trn_agent_boot.trn_boot import _ntff_profile_via_ctypes
        mod = types.ModuleType("antenv.axon_hooks")
        hook = _ntff_profile_via_ctypes("/opt/axon/libaxon_pjrt.so")
        mod.get_axon_ntff_profile_hook = lambda: hook
        mod.set_axon_ntff_profile_hook = lambda h: None
        sys.modules["antenv.axon_hooks"] = mod
        antenv.axon_hooks = mod
    except Exception:
        pass


# ----------------------------------------------------------------------------
# Host preprocessing
# ----------------------------------------------------------------------------

def preprocess(x, edge_index):
    """Build the static SPMD schedule + per-core input arrays."""
    x = np.asarray(x, np.float32)
    ei = np.asarray(edge_index, np.int64)
    # self-loops are NOT placed in the gather stream: each core owns its
    # nodes' table rows, so the self term dinv_i^2*(HW)_i is added on-chip
    # from a stashed copy (identity matmul). deg still counts them.
    src = ei[0]
    dst = ei[1]

    deg = (np.bincount(dst, minlength=N) + 1).astype(np.float32)
    dinv = (1.0 / np.sqrt(deg)).astype(np.float32)

    order = np.argsort(deg, kind="stable")
    rank = np.empty(N, np.int64)
    rank[order] = np.arange(N)
    core_of = rank % NC
    # stratified snake-deal of each core's degree-ordered nodes over its
    # blocks: every block samples the same degree profile, equalizing
    # per-(quad, block) edge counts across blocks AND cores (minimizes
    # gather-tile padding, which is pure wasted gather bandwidth).
    pos = rank // NC
    slot_of = (pos % B) * 128 + pos // B
    gnew = core_of * L + slot_of

    # original node for (core, slot); -1 for dummy slots
    node_at = -np.ones((NC, L), np.int64)
    node_at[core_of, slot_of] = np.arange(N)

    gsrc = gnew[src]
    gdst = gnew[dst]
    owner = gdst // L
    ldst = gdst % L
    # table row numbering: half-shard interleave so the table is assembled
    # by TWO AllGathers (halves) that pipeline with the gather phase.
    #   slot < HALF:  row = core*HALF + slot          (table half A)
    #   slot >= HALF: row = NPAD/2 + core*HALF + slot-HALF   (half B)
    HALF = L // 2                      # 6272 rows (49 blocks) per half-shard
    sc = gsrc // L
    ss = gsrc % L
    trow = np.where(ss < HALF, sc * HALF + ss,
                    NPAD // 2 + sc * HALF + (ss - HALF))
    quad = trow // QROWS
    qidx = trow % QROWS
    blk = ldst // 128
    sid = ldst % 128

    # segment counts per (core, quad, block)
    key = (owner * QUADS + quad) * B + blk
    cnt = np.bincount(key, minlength=NC * QUADS * B).reshape(NC, QUADS, B)
    T = ((cnt + 127) // 128).max(axis=0)          # [QUADS, B] tiles per segment

    # tile schedule: quad-major, block-minor
    tile_q, tile_b = [], []
    seg_tile0 = np.zeros((QUADS, B), np.int64)
    t = 0
    for q in range(QUADS):
        for b in range(B):
            seg_tile0[q, b] = t
            tile_q.extend([q] * int(T[q, b]))
            tile_b.extend([b] * int(T[q, b]))
            t += int(T[q, b])
    tile_q = np.array(tile_q, np.int64)
    tile_b = np.array(tile_b, np.int64)
    n_tiles = t
    S = n_tiles * 128

    # calls: chunk each quad's whole tile range into <=CALL_MAX_TILES-tile
    # calls (pad slots gather valid rows, so calls may span segments).
    calls = []   # (q, tile0, ntiles)
    for q in range(QUADS):
        q_t0 = int(seg_tile0[q, 0])
        q_t1 = int(seg_tile0[q, B - 1] + T[q, B - 1])
        off = q_t0
        while off < q_t1:
            n = min(CALL_MAX_TILES, q_t1 - off)
            calls.append((q, off, n))
            off += n
    n_calls = len(calls)

    # per-block quad participation (static)
    quads_of_b = [[q for q in range(QUADS) if T[q, b] > 0] for b in range(B)]

    # per-core slot arrays; pad slots gather a valid (spread) row but carry
    # sid=-999 so their one-hot column is all zeros. Spread rows avoid HBM
    # hot-row contention and keep every gather tile fully written (needed
    # for both HW determinism and the simulator's ownership model).
    pad_rows = (np.arange(S, dtype=np.int64) * 97) % QROWS
    idx16 = np.tile(pad_rows.astype(np.int16)[None, :], (NC, 1))
    sidf = np.full((NC, S), -999.0, np.float32)

    eorder = np.lexsort((qidx, blk, quad, owner))
    so, sq, sb_, sqi, ssid = (owner[eorder], quad[eorder], blk[eorder],
                              qidx[eorder], sid[eorder])
    skey = key[eorder]
    # within-group rank
    grp_change = np.flatnonzero(np.diff(skey, prepend=-1))
    grp_id = np.cumsum(np.isin(np.arange(len(skey)), grp_change))
    grp_starts = np.zeros(len(skey), np.int64)
    grp_starts[grp_change] = np.arange(len(skey))[grp_change]
    np.maximum.accumulate(grp_starts, out=grp_starts)
    ranks = np.arange(len(skey)) - grp_starts

    slot = seg_tile0[sq, sb_] * 128 + ranks
    idx16[so, slot] = sqi.astype(np.int16)
    sidf[so, slot] = ssid.astype(np.float32)

    callcnt = np.tile(np.array([n * 128 for (_, _, n) in calls],
                               np.int32)[None, :], (NC, 1))

    # wrapped per-core arrays
    idx_wr = np.zeros((NC, 128, S // 16), np.int16)
    sid_wr = np.zeros((NC, 128, S // 128), np.float32)
    for k in range(NC):
        w16 = idx16[k].reshape(S // 16, 16).T            # [16, S/16]
        idx_wr[k] = np.tile(w16, (8, 1))
        sid_wr[k] = sidf[k].reshape(S // 128, 128).T     # [128, S/128]

    # per-core node-major inputs
    xT = np.zeros((NC, 128, L), np.float32)
    dinv_wr = np.zeros((NC, 128, B), np.float32)
    for k in range(NC):
        nodes = node_at[k]
        real = nodes >= 0
        xk = np.zeros((L, C), np.float32)
        xk[real] = x[nodes[real]]
        xT[k] = xk.T
        dk = np.zeros(L, np.float32)
        dk[real] = dinv[nodes[real]]
        dinv_wr[k] = dk.reshape(B, 128).T

    return dict(
        node_at=node_at, dinv=dinv, T=T, S=S, n_tiles=n_tiles,
        tile_q=tile_q, tile_b=tile_b, seg_tile0=seg_tile0,
        calls=calls, n_calls=n_calls, quads_of_b=quads_of_b,
        idx16=idx16, sidf=sidf, callcnt=callcnt,
        idx_wr=idx_wr, sid_wr=sid_wr, xT=xT, dinv_wr=dinv_wr,
    )


def numpy_model(prep, x, Ws, bs, tbl_dt=None):
    """Exact numpy emulation of the device algorithm (for validation)."""
    if tbl_dt is None:
        tbl_dt = TBL_NP
    node_at = prep["node_at"]
    dinv_wr = prep["dinv_wr"]

    # dinv per (core, local) in node-major
    dloc = np.stack([dinv_wr[k].T.reshape(L) for k in range(NC)])   # [NC, L]
    H = np.stack([prep["xT"][k].T for k in range(NC)])              # [NC, L, C]

    out = None
    for l in range(3):
        # table build
        HALF = L // 2
        table = np.zeros((NPAD, C), tbl_dt)
        own = []
        for k in range(NC):
            tk = ((H[k].astype(np.float32) @ Ws[l])
                  * dloc[k][:, None]).astype(tbl_dt)
            own.append(tk)
            table[k * HALF:(k + 1) * HALF] = tk[:HALF]
            table[NPAD // 2 + k * HALF:
                  NPAD // 2 + (k + 1) * HALF] = tk[HALF:]

        # aggregation
        Hn = np.zeros((NC, L, C), np.float32)
        for k in range(NC):
            idx = prep["idx16"][k]
            sidf = prep["sidf"][k]
            S_acc = np.zeros((L, C), np.float32)
            valid = sidf >= 0
            tq = np.repeat(prep["tile_q"], 128)
            tb = np.repeat(prep["tile_b"], 128)
            rows = (prep["idx16"][k][valid].astype(np.int64)
                    + tq[valid] * QROWS)
            tgt = tb[valid] * 128 + sidf[valid].astype(np.int64)
            np.add.at(S_acc, tgt, table[rows].astype(np.float32))
            S_acc += own[k].astype(np.float32)          # self-loop term
            z = S_acc * dloc[k][:, None] + bs[l][None, :]
            Hn[k] = np.maximum(z, 0.0)
        H = Hn
        out = H
    # assemble
    full = np.zeros((N, C), np.float32)
    for k in range(NC):
        real = node_at[k] >= 0
        full[node_at[k][real]] = out[k][real]
    return full


# ----------------------------------------------------------------------------
# Bass program
# ----------------------------------------------------------------------------

def build_nc(prep, tbl_dt_np=None, debug_stage=None):
    import concourse.bass as bass
    import concourse.mybir as mybir
    import concourse.tile as tile
    from concourse import bacc

    if tbl_dt_np is None:
        tbl_dt_np = TBL_NP
    TBL_DT = mybir.dt.from_np(np.dtype(tbl_dt_np))
    F32 = mybir.dt.float32

    S = prep["S"]
    n_tiles = prep["n_tiles"]
    calls = prep["calls"]
    n_calls = prep["n_calls"]
    tile_q = prep["tile_q"]
    tile_b = prep["tile_b"]
    T = prep["T"]
    seg_tile0 = prep["seg_tile0"]
    quads_of_b = prep["quads_of_b"]

    nc = bacc.Bacc("TRN2", target_bir_lowering=False, debug=False,
                   num_devices=NC, num_swdge_queues=N_QUEUES)

    # inputs
    xT_in = nc.dram_tensor("xT", [128, L], F32, kind="ExternalInput")
    w_in = [nc.dram_tensor(f"W{i+1}", [128, 128], F32, kind="ExternalInput")
            for i in range(3)]
    bias_in = [nc.dram_tensor(f"Bt{i+1}", [128, 128], F32, kind="ExternalInput")
               for i in range(3)]
    iota_in = nc.dram_tensor("iota", [128, 128], TBL_DT, kind="ExternalInput")
    ident_in = nc.dram_tensor("ident", [128, 128], F32, kind="ExternalInput")
    identb_in = nc.dram_tensor("identb", [128, 128], TBL_DT,
                               kind="ExternalInput")
    dinv_in = nc.dram_tensor("dinv", [128, B], F32, kind="ExternalInput")
    sid_in = nc.dram_tensor("sid", [128, S // 128], TBL_DT,
                            kind="ExternalInput")
    idx_in = nc.dram_tensor("idx", [128, S // 16], mybir.dt.int16,
                            kind="ExternalInput")
    out_dram = nc.dram_tensor("out", [L, 128], F32, kind="ExternalOutput")
    tbl_dbg_in = None
    slab_dbg = None
    if debug_stage == "agg_only":
        tbl_dbg_in = nc.dram_tensor("tbl_dbg", [NPAD, 128], TBL_DT,
                                    kind="ExternalInput")
        slab_dbg = nc.dram_tensor("slab_dbg", [L, 128], F32,
                                  kind="ExternalOutput")
        g_dbg = nc.dram_tensor("g_dbg", [128, CALL_MAX_TILES * 128], TBL_DT,
                               kind="ExternalOutput")
        a_dbg = nc.dram_tensor("a_dbg", [128, 128], F32,
                               kind="ExternalOutput")

    from contextlib import ExitStack

    with tile.TileContext(nc) as tc, ExitStack() as es:
        constp = es.enter_context(tc.tile_pool(name="const", bufs=1))
        idxp = es.enter_context(tc.tile_pool(name="idxp", bufs=1))
        xtp = es.enter_context(tc.tile_pool(name="xt", bufs=3))
        gatp = es.enter_context(tc.tile_pool(name="gat", bufs=20))
        app = es.enter_context(tc.tile_pool(name="ap", bufs=10))
        slabp = es.enter_context(tc.tile_pool(name="slab", bufs=B))
        workp = es.enter_context(tc.tile_pool(name="work", bufs=4))
        tblp = es.enter_context(tc.tile_pool(name="tblp", bufs=B + 8))
        htp = es.enter_context(tc.tile_pool(name="htp", bufs=2))
        aggps = es.enter_context(tc.tile_pool(name="aggps", bufs=4, space="PSUM"))
        tpps = es.enter_context(tc.tile_pool(name="tpps", bufs=2, space="PSUM"))
        gemmps = es.enter_context(tc.tile_pool(name="gemmps", bufs=2, space="PSUM"))
        dramp = es.enter_context(tc.tile_pool(name="dram", bufs=1, space="DRAM"))
        if True:

            # ---- resident constants ----
            w_sb = []
            bias_sb = []
            for i in range(3):
                w = constp.tile([128, 128], F32, tag=f"w{i}")
                nc.sync.dma_start(w[:], w_in[i][:, :])
                w_sb.append(w)
                bb = constp.tile([128, 128], F32, tag=f"bias{i}")
                nc.sync.dma_start(bb[:], bias_in[i][:, :])
                bias_sb.append(bb)
            iota_sb = constp.tile([128, 128], TBL_DT, tag="iota")
            nc.sync.dma_start(iota_sb[:], iota_in[:, :])
            ident_sb = constp.tile([128, 128], F32, tag="ident")
            nc.sync.dma_start(ident_sb[:], ident_in[:, :])
            identb_sb = constp.tile([128, 128], TBL_DT, tag="identb")
            nc.sync.dma_start(identb_sb[:], identb_in[:, :])
            dinv_sb = constp.tile([128, B], F32, tag="dinv")
            nc.sync.dma_start(dinv_sb[:], dinv_in[:, :])
            sid_sb = constp.tile([128, S // 128], TBL_DT, tag="sid")
            nc.sync.dma_start(sid_sb[:], sid_in[:, :])
            idx_sb = idxp.tile([128, S // 16], mybir.dt.int16, tag="idx")
            nc.sync.dma_start(idx_sb[:], idx_in[:, :])

            HALF = L // 2
            BH = B // 2                # 49 blocks per half
            myshard_a = dramp.tile([HALF, 128], TBL_DT, tag="myshard_a")
            myshard_b = dramp.tile([HALF, 128], TBL_DT, tag="myshard_b")
            # Shared (pair-HBM) tables, one pair per layer: each shared
            # buffer has a single writer (that layer's AllGather), and the
            # cross-core WAR hazard of buffer reuse never arises.
            TBL_SPACE = "Shared" if USE_SHARED_TABLES else "Local"
            tables = [
                (dramp.tile([NPAD // 2, 128], TBL_DT, addr_space=TBL_SPACE,
                            name=f"table_a{p}", tag=f"table_a{p}"),
                 dramp.tile([NPAD // 2, 128], TBL_DT, addr_space=TBL_SPACE,
                            name=f"table_b{p}", tag=f"table_b{p}"))
                for p in range(3)
            ]

            def do_allgather(half, parity):
                shard = myshard_a if half == 0 else myshard_b
                tbl_t = tables[parity][half]
                nc.gpsimd.collective_compute(
                    "AllGather",
                    mybir.AluOpType.bypass,
                    replica_groups=[list(range(NC))],
                    ins=[shard.opt()],
                    outs=[tbl_t.opt()],
                )

            def quad_table_rows(q, parity):
                table_a, table_b = tables[parity]
                if q < 2:
                    return table_a[(q % 2) * QROWS:(q % 2 + 1) * QROWS, :]
                return table_b[(q % 2) * QROWS:(q % 2 + 1) * QROWS, :]

            def myshard_rows(b):
                if b < BH:
                    return myshard_a[b * 128:(b + 1) * 128, :]
                return myshard_b[(b - BH) * 128:(b - BH + 1) * 128, :]

            own_store = {}

            def table_row_block(l, b, lhsT_sb):
                """GEMM + dinv scale (ACT engine) + store to myshard rows."""
                ps = gemmps.tile([128, 128], F32, tag="gemm")
                nc.tensor.matmul(ps[:], lhsT=lhsT_sb[:], rhs=w_sb[l][:],
                                 start=True, stop=True)
                tb = tblp.tile([128, 128], TBL_DT, tag="tbl",
                               name=f"tb_{l}_{b}")
                nc.scalar.activation(tb[:], ps[:],
                                     mybir.ActivationFunctionType.Copy,
                                     bias=0.0, scale=dinv_sb[:, b:b + 1])
                nc.sync.dma_start(myshard_rows(b), tb[:])
                own_store[(l, b)] = tb

            # ---- phase A: layer-1 table from x ----
            if debug_stage == "agg_only":
                nc.sync.dma_start(tables[0][0][:, :], tbl_dbg_in[0:NPAD // 2, :])
                nc.sync.dma_start(tables[0][1][:, :], tbl_dbg_in[NPAD // 2:, :])
            else:
                for b in range(B):
                    xt = xtp.tile([128, 128], F32, tag="xt")
                    nc.sync.dma_start(xt[:], xT_in[:, b * 128:(b + 1) * 128])
                    table_row_block(0, b, xt)
                    if debug_stage != "phaseA" and b == BH - 1:
                        do_allgather(0, 0)
                if debug_stage != "phaseA":
                    do_allgather(1, 0)

            # ---- layers ----
            if debug_stage in ("phaseA", "table1"):
                n_layers = 0
            elif debug_stage in ("layer1", "agg_only"):
                n_layers = 1
            else:
                n_layers = 3
            for l in range(n_layers):
                slabs = [None] * B
                psq = {}
                tails_done = [0, 0]   # per half

                def note_tail_done(b):
                    # fire next layer's half-AllGather once every block of
                    # that half has written its myshard rows
                    half = 0 if b < BH else 1
                    tails_done[half] += 1
                    if (tails_done[half] == BH and l < 2
                            and debug_stage != "agg_only"):
                        do_allgather(half, l + 1)

                def block_tail(b):
                    s = slabs[b]
                    if slab_dbg is not None:
                        nc.sync.dma_start(
                            slab_dbg[b * 128:(b + 1) * 128, :], s[:])
                    u = workp.tile([128, 128], F32, tag="u")
                    nc.vector.scalar_tensor_tensor(
                        u[:], s[:], dinv_sb[:, b:b + 1], bias_sb[l][:],
                        op0=mybir.AluOpType.mult, op1=mybir.AluOpType.add)
                    h = workp.tile([128, 128], F32, tag="h")
                    nc.scalar.activation(h[:], u[:],
                                         mybir.ActivationFunctionType.Relu)
                    if l == 2:
                        nc.sync.dma_start(out_dram[b * 128:(b + 1) * 128, :],
                                          h[:])
                        return
                    tp = tpps.tile([128, 128], F32, tag="tp")
                    nc.tensor.transpose(tp[:], h[:], ident_sb[:])
                    htt = htp.tile([128, 128], F32, tag="ht")
                    nc.scalar.activation(htt[:], tp[:],
                                         mybir.ActivationFunctionType.Copy)
                    table_row_block(l + 1, b, htt)
                    note_tail_done(b)

                for ci, (q, t0, ntl) in enumerate(calls):
                    g = gatp.tile([128, CALL_MAX_TILES, 128], TBL_DT, tag="g")
                    nc.gpsimd.dma_gather(
                        g[:, 0:ntl, :],
                        quad_table_rows(q, l),
                        idx_sb[:, t0 * 8:(t0 + ntl) * 8],
                        ntl * 128, ntl * 128, 128,
                        queue_num=ci % N_QUEUES,
                    )
                    if ci == 0 and debug_stage == "agg_only":
                        nc.sync.dma_start(
                            g_dbg[:, 0:ntl * 128],
                            g[:, 0:ntl, :].rearrange("p t f -> p (t f)"))
                    a_all = app.tile([128, CALL_MAX_TILES, 128], TBL_DT,
                                     tag="a")
                    iota3 = iota_sb[:].rearrange("p (o f) -> p o f", o=1)
                    sid3 = sid_sb[:, t0:t0 + ntl].rearrange(
                        "p (t o) -> p t o", o=1)
                    i_b, s_b = bass.broadcast_tensor_aps(iota3, sid3)
                    nc.vector.tensor_tensor(a_all[:, 0:ntl, :], i_b, s_b,
                                            op=mybir.AluOpType.is_equal)
                    for tl in range(ntl):
                        gt = t0 + tl
                        b = int(tile_b[gt])
                        a = a_all[:, tl, :]
                        if ci == 0 and tl == 0 and debug_stage == "agg_only":
                            nc.sync.dma_start(a_dbg[:, :], a)
                        first = (gt == seg_tile0[q, b])
                        last = (gt == seg_tile0[q, b] + T[q, b] - 1)
                        if first:
                            psq[b] = aggps.tile([128, 128], F32, tag="agg", name=f"agg_{l}_{q}_{b}")
                        do_self = (first and q == quads_of_b[b][0]
                                   and (l, b) in own_store)
                        nc.tensor.matmul(psq[b][:], lhsT=a,
                                         rhs=g[:, tl, :],
                                         start=first,
                                         stop=last and not do_self)
                        if do_self:
                            # self-loop term: psum += I^T @ own_rows
                            nc.tensor.matmul(psq[b][:], lhsT=identb_sb[:],
                                             rhs=own_store[(l, b)][:],
                                             start=False, stop=last)
                        if last:
                            qs = quads_of_b[b]
                            if q == qs[0]:
                                slabs[b] = slabp.tile([128, 128], F32, tag="slab", name=f"slab_{l}_{b}")
                                nc.scalar.activation(
                                    slabs[b][:], psq[b][:],
                                    mybir.ActivationFunctionType.Copy)
                            else:
                                nc.vector.tensor_tensor(
                                    slabs[b][:], slabs[b][:], psq[b][:],
                                    op=mybir.AluOpType.add)
                            if q == qs[-1]:
                                block_tail(b)

    nc.compile()
    return nc


# ----------------------------------------------------------------------------
# Runner
# ----------------------------------------------------------------------------

def make_in_maps(prep, Ws, bs):
    iota = np.tile(np.arange(128, dtype=np.float32)[None, :], (128, 1))
    ident = np.eye(128, dtype=np.float32)
    maps = []
    for k in range(NC):
        maps.append({
            "xT": prep["xT"][k],
            "W1": Ws[0].astype(np.float32),
            "W2": Ws[1].astype(np.float32),
            "W3": Ws[2].astype(np.float32),
            "Bt1": np.tile(bs[0][None, :], (128, 1)).astype(np.float32),
            "Bt2": np.tile(bs[1][None, :], (128, 1)).astype(np.float32),
            "Bt3": np.tile(bs[2][None, :], (128, 1)).astype(np.float32),
            "iota": iota.astype(TBL_NP),
            "ident": ident,
            "identb": ident.astype(TBL_NP),
            "dinv": prep["dinv_wr"][k],
            "sid": prep["sid_wr"][k].astype(TBL_NP),
            "idx": prep["idx_wr"][k],
        })
    return maps


def assemble_output(prep, results):
    full = np.zeros((N, C), np.float32)
    for k in range(NC):
        nodes = prep["node_at"][k]
        real = nodes >= 0
        full[nodes[real]] = results[k]["out"][real]
    return full


_CACHE = {}


def run(inputs, trace=False, sim=False):
    from concourse.bass_utils import run_bass_kernel_spmd

    x = np.asarray(inputs["x"], np.float32)
    Ws = [np.asarray(inputs[f"W{i+1}"], np.float32) for i in range(3)]
    bs = [np.asarray(inputs[f"b{i+1}"], np.float32) for i in range(3)]

    prep = preprocess(x, inputs["edge_index"])
    ckey = ("nc", TBL_NP, prep["S"], prep["n_calls"])
    if ckey not in _CACHE:
        _CACHE[ckey] = build_nc(prep)
    nc = _CACHE[ckey]

    in_maps = make_in_maps(prep, Ws, bs)

    if sim:
        from concourse.bass_interp import MultiCoreSim
        msim = MultiCoreSim(nc, NC, trace=False, require_finite=False,
                            require_nnan=False)
        for k in range(NC):
            for name, arr in in_maps[k].items():
                msim.cores[k].tensor(name)[:] = arr
        msim.simulate(check_with_hw=False)
        results = [{"out": np.array(msim.cores[k].tensor("out"))}
                   for k in range(NC)]
        return assemble_output(prep, results), None

    if trace:
        _install_axon_profile_hook()
    res = run_bass_kernel_spmd(nc, in_maps, list(range(NC)), trace=trace)
    return assemble_output(prep, res.results), res


def kernel(**inputs):
    out, _ = run(inputs)
    return out

